# revision 1
# baseline (speedup 1.0000x reference)
"""Trainium2 Bass kernel for the kNN pairwise-ranking loss.

Math: with y = (knn_tgts == tgts), the masked pairwise BCE-with-logits loss
over differing-label pairs (j > i) collapses to

    loss = sum_b sum_{n in neg_b} sum_{p in pos_b} softplus(s_n - s_p) / cnt
    cnt  = sum_b |pos_b| * |neg_b|

because for a (pos, neg) pair the per-pair term is softplus(s_neg - s_pos)
regardless of orientation, and b2 cancels in score differences.

Host side: per batch row, permute keys so positives come first, then
negatives.  Rows are sorted by positive count and dealt to (core, slot) so
each slot's compile-time widths are tight.  Additive kill values (+-200 on
invalid lanes) make killed exponentials underflow to exactly 0.

Device (SPMD over 8 cores, 4 batch rows each):
  phase A (per row): h = relu(W1 @ keys^T + b1) via PE (fp8 DoubleRow, f32
                     psum).  hh is augmented with two DMA'd pad rows
                     (npad, ppad) at partitions 100:102, so the two score
                     matmuls (lhsT [w2;1;0] and [-w2;0;-1]) emit
                     s_n + npad (cols nst:1024, PSUM p100) and
                     -(s_p + ppad) (shifted to cols nst:1024, PSUM p101)
                     with the kills pre-folded -- no DVE pad work at all.
  phase B (per row): two ACT Exp passes produce e^{s_n'} [1,LTw] at T[0]
                     and e^{-s_p'} [1,Pw] at T[64] (both legal quadrant
                     starts); contraction-1 outer-product matmuls form
                     e^{d} in PSUM; one ACT Ln(1+x) pass per 2-chunk group
                     with accum_out yields per-partition sums.  Exp and Ln
                     share one ACT table set (natural_log_exp_and_others).
Host gathers [128, NC] partial sums, reduces, divides by cnt.
"""

import numpy as np

B, K, D, H = 32, 1024, 1024, 100
N_CORES = 8
BPC = B // N_CORES  # batch rows per core
KILL = 200.0

_cache = {}
_act_patched = False


def _patch_act_tables():
    """Make Exp/Ln resolve to the single combined ACT table set."""
    global _act_patched
    if _act_patched:
        return
    import concourse.bacc as bacc
    import concourse.hw_specs as hw_specs
    import concourse.mybir as mybir

    orig = hw_specs.get_activation_tables
    combined = "natural_log_exp_and_others"

    def patched(arch):
        tabs = orig(arch)
        out = {}
        for name, funcs in tabs.items():
            f = set(funcs)
            if name != combined and combined in tabs:
                f.discard(mybir.ActivationFunctionType.Exp)
                f.discard(mybir.ActivationFunctionType.Ln)
            out[name] = f
        return out

    hw_specs.get_activation_tables = patched
    bacc.get_activation_tables = patched
    _act_patched = True


def _build_program(slot_params):
    """slot_params: tuple of (Pw, nch) per slot; nst = 1024 - 128*nch."""
    import concourse.bacc as bacc
    import concourse.mybir as mybir
    import concourse.tile as tile

    _patch_act_tables()

    f32 = mybir.dt.float32
    bf16 = mybir.dt.bfloat16
    fp8 = mybir.dt.float8e4
    ndc = 4  # contraction chunks (256 wide with DoubleRow)
    hpad = 112  # padded per-subrow weight stride (DoubleRow needs step%16==0)
    wchunk = 2 * hpad

    # per-slot Ln call layout: chunks-per-bank and parts (each part is a
    # run of chunks filling the single 4-bank op tile, one Ln call each)
    slot_groups = []
    ncalls = 0
    for Pw, nch in slot_params:
        cpb = max(1, 512 // Pw)  # outer-product chunks packed per PSUM bank
        per_tile = 3 * cpb  # op tile = 3 banks
        parts = []
        c0 = 0
        while c0 < nch:
            n = min(per_tile, nch - c0)
            parts.append((c0, n))
            c0 += n
        slot_groups.append((cpb, parts))
        ncalls += len(parts)

    nc = bacc.Bacc(
        "TRN2",
        target_bir_lowering=False,
        debug=False,
        enable_asserts=False,
        num_devices=N_CORES,
    )

    keys_d = nc.dram_tensor(
        "keys_t", [BPC, 128, ndc * 2 * K], fp8, kind="ExternalInput"
    ).ap()
    w1t_d = nc.dram_tensor("w1t", [128, ndc * wchunk], fp8, kind="ExternalInput").ap()
    w2a_d = nc.dram_tensor("w2a", [H + 2, 2], bf16, kind="ExternalInput").ap()
    b1_d = nc.dram_tensor("b1c", [H, 1], f32, kind="ExternalInput").ap()
    pads_d = nc.dram_tensor("pads", [BPC, 6, K], bf16, kind="ExternalInput").ap()
    out_d = nc.dram_tensor("acc_out", [128, ncalls], f32, kind="ExternalOutput").ap()

    with tile.TileContext(nc) as tc:
        with (
            tc.tile_pool(name="const", bufs=1) as cpool,
            tc.tile_pool(name="keys", bufs=2 * BPC + 2) as kpool,
            tc.tile_pool(name="h", bufs=BPC) as hpool,
            tc.tile_pool(name="T1", bufs=2) as t1pool,
            tc.tile_pool(name="T2", bufs=2) as t2pool,
            tc.tile_pool(name="hp", bufs=2, space="PSUM") as hp_pool,
            tc.tile_pool(name="op", bufs=2, space="PSUM") as op_pool,
        ):
            # DMA issue order is load-bearing: HWDGE completion semaphores
            # aggregate by issue order, so small constants interleave with
            # the first keys chunks and the bulk keys stream goes last.
            acc_sb = cpool.tile([128, ncalls], f32, tag="acc")
            dummy_sb = cpool.tile([128, 2048], bf16, tag="dummy")

            # row-0 keys as quarter-DMAs so the first matmul starts sooner
            kt00 = kpool.tile([128, 2 * K], fp8, tag="keys")
            nc.sync.dma_start(kt00[:], keys_d[0, :, 0 : 2 * K])
            w1t_sb = cpool.tile([128, ndc * wchunk], fp8, tag="w1t")
            nc.scalar.dma_start(w1t_sb[:], w1t_d[:])
            kt01 = kpool.tile([128, 2 * K], fp8, tag="keys")
            nc.sync.dma_start(kt01[:], keys_d[0, :, 2 * K : 4 * K])
            w2a_sb = cpool.tile([H + 2, 2], bf16, tag="w2a")
            nc.scalar.dma_start(w2a_sb[:], w2a_d[:])
            b1_sb = cpool.tile([H, 1], f32, tag="b1")
            nc.scalar.dma_start(b1_sb[:], b1_d[:])
            kt02 = kpool.tile([128, 2 * K], fp8, tag="keys")
            nc.sync.dma_start(kt02[:], keys_d[0, :, 4 * K : 6 * K])
            kt03 = kpool.tile([128, 2 * K], fp8, tag="keys")
            nc.sync.dma_start(kt03[:], keys_d[0, :, 6 * K : 8 * K])

            # hh tiles (one per row): pad rows DMA'd up front on the SWDGE
            # path.  Rows 0:100 = relu(h) written later, 100 = npad,
            # 101 = ppad; the DMA covers junk rows 96:100 (quadrant rule).
            hhs = []
            for r in range(BPC):
                hh = hpool.tile([H + 2, K], bf16, tag="h")
                nc.gpsimd.dma_start(hh[96 : H + 2, :], pads_d[r, :, :])
                hhs.append(hh)

            # ---- bulk keys stream on the sync queue ----
            # per row: 4 (tile, col0) accessors, one per dc chunk
            kq = [[(kt00, 0), (kt01, 0), (kt02, 0), (kt03, 0)]]
            for r in range(1, BPC):
                quads = []
                for hf in range(2):
                    kt = kpool.tile([128, ndc * K], fp8, tag="keys")
                    nc.sync.dma_start(
                        kt[:], keys_d[r, :, hf * ndc * K : (hf + 1) * ndc * K]
                    )
                    quads += [(kt, 0), (kt, 2 * K)]
                kq.append(quads)

            scs = [None] * BPC
            state = {"call": 0}

            def stageAmm(r):
                hh = hhs[r]
                # two 1-bank PSUM tiles with dc-matmuls interleaved between
                # them: back-to-back accumulation into one bank runs at half
                # rate, alternating banks pipelines at full rate.
                hp0 = hp_pool.tile([H, 512], f32, tag="hpx")
                hp1 = hp_pool.tile([H, 512], f32, tag="hpx")
                hps = [hp0, hp1]
                for dc in range(ndc):
                    kt, c0 = kq[r][dc]
                    kt3 = kt[:, c0 : c0 + 2 * K].rearrange(
                        "p (i k) -> p i k", i=2
                    )
                    w_sl = w1t_sb[
                        :, dc * wchunk : (dc + 1) * wchunk
                    ].rearrange("p (i m) -> p i m", i=2)[:, :, 0:H]
                    for kh in range(2):
                        nc.tensor.matmul(
                            hps[kh][:, :],
                            lhsT=w_sl,
                            rhs=kt3[:, :, kh * 512 : (kh + 1) * 512],
                            start=(dc == 0),
                            stop=(dc == ndc - 1),
                            perf_mode=mybir.MatmulPerfMode.DoubleRow,
                        )
                for kh in range(2):
                    # relu(h + b1): PSUM -> SBUF on DVE (bf16 for scores)
                    nc.vector.tensor_scalar(
                        hh[0:H, kh * 512 : (kh + 1) * 512],
                        hps[kh][:, :],
                        b1_sb[:],
                        0.0,
                        op0=mybir.AluOpType.add,
                        op1=mybir.AluOpType.max,
                    )

            def stageScore(r):
                Pw, nch = slot_params[r]
                nst = K - 128 * nch
                hh = hhs[r]
                # score rows: S[0] = s + npad (full K);
                # S[32] = -(s + ppad) for p in [0,Pw), shifted to cols
                # nst:nst+Pw.  S shares the op pool rotation (2 of 3 banks).
                S = op_pool.tile([64, 3 * 512], f32, tag="op")
                scs[r] = S
                # -s' first: needs only the kh0 half of relu (cols 0:Pw),
                # letting exp#2 start before the kh1 relu lands.  Segments
                # split at the PSUM bank boundary (col 512).
                segs = [(nst, min(512, nst + Pw))]
                if nst + Pw > 512:
                    segs.append((512, nst + Pw))
                for o0, o1 in segs:
                    nc.tensor.matmul(
                        S[32:33, o0:o1],
                        lhsT=w2a_sb[:, 1:2],
                        rhs=hh[:, o0 - nst : o1 - nst],
                        start=True,
                        stop=True,
                    )
                for kh in range(2):
                    nc.tensor.matmul(
                        S[0:1, kh * 512 : (kh + 1) * 512],
                        lhsT=w2a_sb[:, 0:1],
                        rhs=hh[:, kh * 512 : (kh + 1) * 512],
                        start=True,
                        stop=True,
                    )

            def stageExp(r):
                Pw, nch = slot_params[r]
                nst = K - 128 * nch
                # positive side first -- its scores are ready earlier
                T2 = t2pool.tile([1, 512], bf16, tag="T2")
                nc.scalar.activation(
                    T2[0:1, 0:Pw],
                    scs[r][32:33, nst : nst + Pw],
                    mybir.ActivationFunctionType.Exp,
                    scale=1.0,
                )
                T1 = t1pool.tile([1, K], bf16, tag="T1")
                nc.scalar.activation(
                    T1[0:1, 0 : 128 * nch],
                    scs[r][0:1, nst:K],
                    mybir.ActivationFunctionType.Exp,
                    scale=1.0,
                )
                return T1, T2

            def _ln_call(ap_src, ap_dst):
                nc.scalar.activation(
                    ap_dst,
                    ap_src,
                    mybir.ActivationFunctionType.Ln,
                    bias=1.0,
                    scale=1.0,
                    accum_out=acc_sb[:, state["call"] : state["call"] + 1],
                )
                state["call"] += 1

            def emit_part(r, T1, T2, c0, n):
                """Outer-product mms for chunks c0..c0+n plus one Ln call."""
                Pw, nch = slot_params[r]
                cpb, _ = slot_groups[r]
                bw = 512 // cpb  # per-chunk column stride within a bank
                rhs = T2[0:1, 0:Pw]
                op = op_pool.tile([128, 3 * 512], f32, tag="op")
                for j in range(n):
                    col = j * bw
                    nc.tensor.matmul(
                        op[:, col : col + Pw],
                        lhsT=T1[0:1, (c0 + j) * 128 : (c0 + j + 1) * 128],
                        rhs=rhs,
                        start=True,
                        stop=True,
                    )
                if n == 1:
                    src, dst = op[:, 0:Pw], dummy_sb[:, 0:Pw]
                else:
                    src = op[:, 0 : n * bw].rearrange("p (i w) -> p i w", i=n)[
                        :, :, 0:Pw
                    ]
                    dst = dummy_sb[:, 0 : n * bw].rearrange(
                        "p (i w) -> p i w", i=n
                    )[:, :, 0:Pw]
                _ln_call(src, dst)

            stageAmm(0)
            stageAmm(1)
            stageScore(0)
            Ts = {0: stageExp(0)}
            for r in range(BPC):
                parts = slot_groups[r][1]
                T1, T2 = Ts.pop(r)
                if r == 0:
                    # start-up: row-1's scores aren't ready yet; issuing
                    # exp(1) first would block ready Ln(0) work on ACT
                    emit_part(r, T1, T2, *parts[0])
                    stageAmm(2)
                    stageScore(1)
                    Ts[1] = stageExp(1)
                    for c0, n in parts[1:]:
                        emit_part(r, T1, T2, c0, n)
                else:
                    # drain: exps feed everything -- issue them first
                    if r + 2 < BPC:
                        stageAmm(r + 2)
                    if r + 1 < BPC:
                        stageScore(r + 1)
                        Ts[r + 1] = stageExp(r + 1)
                    for c0, n in parts:
                        emit_part(r, T1, T2, c0, n)

            nc.sync.dma_start(out_d[:], acc_sb[:])

    nc.compile()
    return nc, state["call"]


def kernel(keys, tgts, knn_tgts, mask, W1, b1, W2, b2, _profile=False):
    import ml_dtypes

    from concourse.bass_utils import run_bass_kernel_spmd

    keys = np.asarray(keys, dtype=np.float32)
    tgts = np.asarray(tgts)
    knn_tgts = np.asarray(knn_tgts)
    mask = np.asarray(mask).astype(bool)
    W1 = np.asarray(W1, dtype=np.float32)
    b1 = np.asarray(b1, dtype=np.float32)
    W2 = np.asarray(W2, dtype=np.float32)

    # ---- host-side label/permutation prep ----
    y = knn_tgts == tgts[:, None]
    pos = y & mask
    neg = (~y) & mask
    P = pos.sum(axis=1).astype(np.int64)
    N_ = neg.sum(axis=1).astype(np.int64)
    cnt = float((P * N_).sum())

    # stable order: positives, negatives, masked-out
    rank = np.where(pos, 0, np.where(neg, 1, 2)).astype(np.int8)
    order = np.argsort(rank, axis=1, kind="stable")  # [B, K]

    # deal rows sorted by P desc: rank i -> core i%8, slot i//8
    rows_by_p = np.argsort(-P, kind="stable")
    assign = rows_by_p.reshape(BPC, N_CORES)  # [slot, core] -> row id

    slot_params = []
    for r in range(BPC):
        ps = P[assign[r]]
        Pw = int(ps.max())
        nch = (K - int(ps.min()) + 127) // 128
        slot_params.append((Pw, nch))
    slot_params = tuple(slot_params)

    # permuted, transposed keys in fp8 DoubleRow layout: per row [128, 8K]
    # free index = dc*2048 + i*1024 + k  (contraction d = dc*256 + i*128 + p)
    keys_perm = np.take_along_axis(keys, order[:, :, None], axis=1)  # [B, K, D]
    kt = keys_perm.transpose(0, 2, 1).astype(ml_dtypes.float8_e4m3)  # [B, D, K]
    kt = np.ascontiguousarray(
        kt.reshape(B, 4, 2, 128, K).transpose(0, 3, 1, 2, 4).reshape(B, 128, 8 * K)
    )

    # scale W1 by 16 into fp8's sweet spot; fold 1/16 into W2 and 16 into b1
    hpad = 112
    ndc = 4
    w1s = (W1.T * 16.0).astype(np.float32)  # [D, H]
    w4 = np.zeros((ndc, 2, 128, hpad), dtype=np.float32)
    w4[:, :, :, :H] = w1s.reshape(ndc, 2, 128, H)
    w1t = np.ascontiguousarray(
        w4.transpose(2, 0, 1, 3).reshape(128, ndc * 2 * hpad)
    ).astype(ml_dtypes.float8_e4m3)
    w2v = (W2.reshape(H) / 16.0).astype(np.float32)
    w2a = np.zeros((H + 2, 2), dtype=np.float32)
    w2a[:H, 0] = w2v
    w2a[H, 0] = 1.0  # + npad row
    w2a[:H, 1] = -w2v
    w2a[H + 1, 1] = -1.0  # - ppad row
    w2a = w2a.astype(ml_dtypes.bfloat16)
    b1c = np.ascontiguousarray(b1.reshape(H, 1) * 16.0)

    # pad rows riding in hh partitions 100:102 (shipped at rows 4:6 of a
    # 6-row block whose rows 0:4 land on junk partitions 96:100):
    #   npad[j] = -KILL for j < P  (kills positives on the negative side)
    #   ppad[j] = +KILL for j >= P (kills non-positives on the positive side)
    kidx = np.arange(K)[None, :]
    pads = np.zeros((N_CORES, BPC, 6, K), dtype=np.float32)
    for r in range(BPC):
        pr = P[assign[r]][:, None]  # [cores, 1]
        pads[:, r, 4, :] = np.where(kidx < pr, -KILL, 0.0)
        pads[:, r, 5, :] = np.where(kidx < pr, 0.0, KILL)
    pads = pads.astype(ml_dtypes.bfloat16)

    key = slot_params
    if key not in _cache:
        _cache[key] = _build_program(slot_params)
    nc, ncalls = _cache[key]

    in_maps = []
    for c in range(N_CORES):
        in_maps.append(
            {
                "keys_t": np.ascontiguousarray(kt[assign[:, c]]),
                "w1t": w1t,
                "w2a": w2a,
                "b1c": b1c,
                "pads": np.ascontiguousarray(pads[c]),
            }
        )

    res = run_bass_kernel_spmd(
        nc, in_maps, list(range(N_CORES)), trace=bool(_profile)
    )
    total = 0.0
    for r in res.results:
        total += float(r["acc_out"].astype(np.float64).sum())
    if _profile:
        print(f"HW exec time: {res.exec_time_ns} ns")
        globals()["_last_results"] = res
    loss = np.float64(total) / np.float64(cnt)
    return np.array(loss, dtype=np.float32)

